# revision 35
# baseline (speedup 1.0000x reference)
"""Trainium2 Bass kernel for filtered backprojection (FBP).

reference semantics:
    filtered = irfft(rfft(sinos, axis=-1) * kernel, n=512, axis=-1)
    out[b,i,j] = sum_phi lerp(filtered[b,phi,:], u(phi,i,j)) * DPHI
with u affine in (i,j) per angle.

Device pipeline (8 NeuronCores, SPMD):
  F0  h = irfft(kernel) via small f32r matmuls against a host irfft matrix
  F1  circulant C[s,t] = h[(t-s)%512] built via per-partition indirect DMA
  F2  filter: filtered rows = sinoT.T @ C (bf16 matmuls), angle-sharded
  F3  store filt rows bf16 to DRAM in two 45-angle halves
  F4  two AllGathers (one per half, overlapping F2/F3 of the other half);
      filt_full row layout is [half, rank, 45, t, b]
  B   backprojection: image in 16x16 tiles; per (angle,tile) only a 32-wide
      detector window contributes (exact: span <= 15*(|A|+|B|)+2 <= 32).
      D4 symmetry (8 exact pixel-grid symmetries) dedups weight blocks 8x.
      Per canonical tile nu: 16 per-(class-chunk, slot) indirect DMAs fetch
      aligned windows (one int32 tap-row offset per class partition); a
      vector/scalar-engine repack interleaves the 8 slots into lhsT layout
      [cls, gh, cch, tap, (4 slots x 32 b)]; 128 matmuls (M=128 stationary
      = 4 same-sign group elements x batch, N=256 pixels) accumulate two
      psum tiles; slots are DMA'd out and the sigma-permuted merge happens
      on host.

Weights/idx tables are pure geometry -> precomputed on host in fp64.
"""
import numpy as np
import ml_dtypes

# ---------------- geometry constants ----------------
PHI, T, H, W = 720, 512, 256, 256
RHO = float(np.sqrt(2.0))
DPHI = float(np.pi) / PHI
DT = 2.0 * RHO / T
T0 = -RHO + 0.5 * DT
DX = 2.0 / H
TS, NT = 16, 16            # tile size / tiles per side
KWIN = 32                  # weight k-window (taps per (class,tile)); exact:
                           # span <= 15*(|A|+|B|) + 2 <= 32, base = floor(min u)
GWIN = 32                  # gathered k-window == KWIN
NCLS = 180                 # D4 angle classes
NG = 8                     # group size
NCORE = 8
B = 32
NB_LOC = 4                 # batch per core is not used (angle-sharded filter)
APC = PHI // NCORE         # angles per core for the filter stage (90)
NU_PER_CORE = 16           # canonical-tile units per core
NGRP = 4                   # nu groups per core (4 nu each)

# ---------------- D4 group tables ----------------
def _mats():
    out = []
    for swap in (False, True):
        for sx in (1, -1):
            for sy in (1, -1):
                if not swap:
                    out.append(np.array([[sx, 0], [0, sy]]))
                else:
                    out.append(np.array([[0, sx], [sy, 0]]))
    return out

MATS = _mats()

def _angle_dirs():
    th = (np.arange(PHI) + 0.5) * DPHI
    return np.stack([np.cos(th), np.sin(th)], axis=1)

def _angle_maps():
    dirs = _angle_dirs()
    amap = np.zeros((NG, PHI), np.int64)
    smap = np.zeros((NG, PHI), np.int64)
    for gi, M in enumerate(MATS):
        v = dirs @ M          # row a: M^T d(a)
        dots = v @ dirs.T     # [a, a']
        ip = np.argmax(dots, axis=1)
        im = np.argmin(dots, axis=1)
        for a in range(PHI):
            if dots[a, ip[a]] > 1 - 1e-9:
                amap[gi, a], smap[gi, a] = ip[a], 1
            elif dots[a, im[a]] < -1 + 1e-9:
                amap[gi, a], smap[gi, a] = im[a], -1
            else:
                raise AssertionError((gi, a))
    inv_a = np.zeros_like(amap); inv_s = np.zeros_like(smap)
    for gi in range(NG):
        inv_a[gi, amap[gi]] = np.arange(PHI)
        inv_s[gi, amap[gi]] = smap[gi]
    return amap, smap, inv_a, inv_s

def _pixel_map(M):
    def f(i, j):
        def comp(row, i, j):
            if row[0] == 1:  return i
            if row[0] == -1: return 255 - i
            if row[1] == 1:  return j
            return 255 - j
        return comp(M[0], i, j), comp(M[1], i, j)
    return f

def _tile_pixel_maps():
    gT, gP = [], []
    for M in MATS:
        f = _pixel_map(M)
        tm = np.zeros(NT * NT, np.int64)
        pm = np.zeros(TS * TS, np.int64)
        for ti in range(NT):
            for tj in range(NT):
                a1 = f(ti * TS, tj * TS)
                a2 = f(ti * TS + TS - 1, tj * TS + TS - 1)
                tm[ti * NT + tj] = (min(a1[0], a2[0]) // TS) * NT + min(a1[1], a2[1]) // TS
        for pi in range(TS):
            for pj in range(TS):
                i_, j_ = f(pi, pj)
                pm[pi * TS + pj] = (i_ % TS) * TS + (j_ % TS)
        gT.append(tm); gP.append(pm)
    return gT, gP

def _uABC(a):
    th = (np.asarray(a, np.float64) + 0.5) * DPHI
    A = DX * np.cos(th) / DT
    Bc = DX * np.sin(th) / DT
    C = ((-1 + 0.5 * DX) * (np.cos(th) + np.sin(th)) - T0) / DT
    return A, Bc, C

_plan_cache = None

def build_plan():
    global _plan_cache
    if _plan_cache is not None:
        return _plan_cache
    amap, smap, inv_a, inv_s = _angle_maps()
    gT, gP = _tile_pixel_maps()
    rho_i = next(gi for gi, M in enumerate(MATS)
                 if M[0][0] == -1 and M[0][1] == 0 and M[1][1] == -1)
    assert all(amap[rho_i, a] == a and smap[rho_i, a] == -1 for a in range(PHI))

    # per-g uniform sign + band of the inverse angle map over classes 0..179
    bands = np.zeros(NG, np.int64)
    signs = np.zeros(NG, np.int64)
    for gi in range(NG):
        aa = inv_a[gi, :NCLS]
        ss = inv_s[gi, :NCLS]
        assert (ss == ss[0]).all(), gi
        bd = aa // NCLS
        assert (bd == bd[0]).all(), (gi, np.unique(bd))
        bands[gi] = bd[0]
        signs[gi] = ss[0]
        # relative angle within band must equal a function of c; store per-class
    # classes: verify canonical range
    for a in range(NCLS):
        assert min(set(amap[:, a])) == a or True

    # tile orbits under D4; canonical taus: tau < rho180(tau)
    rho_t = gT[rho_i]
    tseen = np.zeros(NT * NT, bool)
    orbits = []
    for t in range(NT * NT):
        if tseen[t]: continue
        tiles = []
        for gi in range(NG):
            tt = gT[gi][t]
            if not tseen[tt]:
                tseen[tt] = True
                tiles.append(tt)
        orbits.append(tiles)
    canon = [[t for t in tiles if t < rho_t[t]] for tiles in orbits]
    # schedule: cores 0-6 get four size-8 orbits (4 canon each);
    # core 7 gets the eight size-4 orbits (2 canon each)
    big = [i for i, o in enumerate(orbits) if len(o) == 8]
    small = [i for i, o in enumerate(orbits) if len(o) == 4]
    assert len(big) == 28 and len(small) == 8, (len(big), len(small))
    core_nus = []   # per core: list of 16 canonical tile ids
    for c in range(7):
        nus = []
        for oi in big[c * 4:(c + 1) * 4]:
            nus.extend(canon[oi])
        assert len(nus) == 16
        core_nus.append(nus)
    nus7 = []
    for oi in small:
        nus7.extend(canon[oi])
    assert len(nus7) == 16
    core_nus.append(nus7)

    # per (class, tile): fp64 window base (4-aligned), weights
    ii = np.arange(TS, dtype=np.float64)
    def u_tile(a, tile):
        ti, tj = divmod(int(tile), NT)
        A, Bc, C = _uABC(a)
        return (A * (ti * TS + ii[:, None]) + Bc * (tj * TS + ii[None, :]) + C)

    def base_of(a, tile):
        u = u_tile(a, tile)
        b = int(np.floor(u.min()))
        return int(np.clip(b, 0, T - KWIN))

    # gi slot order: slots 0-3 = sign +1 (bands 0..3), slots 4-7 = sign -1
    # (bands 0..3); lets one lhsT [cls, (4 gi, 32 b)] span a sign-uniform
    # group half with a single tap index.
    order = sorted(range(NG), key=lambda gi: (0 if signs[gi] == 1 else 1,
                                              bands[gi]))
    assert all(signs[g] == 1 for g in order[:4])
    assert all(signs[g] == -1 for g in order[4:])
    assert [int(bands[g]) for g in order[:4]] == [0, 1, 2, 3]
    assert [int(bands[g]) for g in order[4:]] == [0, 1, 2, 3]

    # W tables per core: WA [16,128,KWIN,256], WB [16,52,KWIN,256] bf16
    # gather offsets per core: [16, 2(cch), 128, 8(slot)] int32 global tap-row
    # indices into filt_full (row = 32 batch elems; value*32 = elem offset)
    # merge spec per core: [16, 8(slot)] -> (tile m, sigma gi)  (m from gT)
    kk = np.arange(KWIN, dtype=np.float64)
    WAs, WBs, GIs, merges = [], [], [], []
    for c in range(NCORE):
        Wfull = np.zeros((NU_PER_CORE, NCLS, KWIN, TS * TS), np.float64)
        gidx = np.zeros((NU_PER_CORE, 2, 128, NG), np.int32)
        mspec = np.zeros((NU_PER_CORE, NG, 2), np.int64)
        for j, tau in enumerate(core_nus[c]):
            U = np.stack([u_tile(cc, tau).reshape(-1) for cc in range(NCLS)])  # [180,256]
            b0 = np.array([base_of(cc, tau) for cc in range(NCLS)])            # [180]
            rel = U[:, None, :] - b0[:, None, None] - kk[None, :, None]
            Wk = np.maximum(0.0, 1.0 - np.abs(rel))
            tap = b0[:, None] + kk[None, :]
            Wk[(tap < 0) | (tap >= T), :] = 0.0
            Wfull[j] = Wk * DPHI
            for slot in range(NG):
                gi = order[slot]
                s = signs[gi]
                aa = inv_a[gi, :NCLS]          # member angles [180]
                arel = aa % NCLS
                mb = b0 if s == 1 else (T - KWIN) - b0
                # filt_full row layout: [half(2), rank(8), 45, t] of 32-elem
                # tap-rows, matching the two per-half rank-major AllGathers
                phi = aa                        # absolute member angle [720]
                rr, ap90 = phi // 90, phi % 90
                hh, a45 = ap90 // 45, ap90 % 45
                idxv = ((hh * NCORE + rr) * 45 + a45) * T + mb
                for cc in range(NCLS):
                    gidx[j, cc // 128, cc % 128, slot] = idxv[cc]
                mspec[j, slot] = (gT[gi][tau], gi)
        WAs.append(Wfull[:, :128].astype(ml_dtypes.bfloat16))
        WBs.append(Wfull[:, 128:].astype(ml_dtypes.bfloat16))
        GIs.append(gidx)
        merges.append(mspec)

    # irfft matrix [257, 512] fp64->fp32: h[t] = sum_f IRm[f, t] * k[f]
    tt = np.arange(T)
    ff = np.arange(T // 2 + 1)
    IRm = 2.0 * np.cos(2 * np.pi * np.outer(ff, tt) / T) / T
    IRm[0] *= 0.5
    IRm[T // 2] *= 0.5
    IRm = IRm.astype(np.float32)

    # circulant build offsets: C[s, t] = h2[512 - s + t]; per s-chunk [128,1]
    coffs = np.zeros((4, 128, 1), np.int32)
    for ch in range(4):
        for p in range(128):
            coffs[ch, p, 0] = 512 - (128 * ch + p)

    _plan_cache = dict(
        inv_a=inv_a, inv_s=inv_s, signs=signs, bands=bands, order=order,
        gT=gT, gP=gP, core_nus=core_nus,
        WAs=WAs, WBs=WBs, GIs=GIs, merges=merges,
        IRm=IRm, coffs=coffs,
    )
    return _plan_cache


def host_reference_from_plan(sinos, kern):
    """Numpy simulation of the EXACT device pipeline (bf16 quantization
    included) for validating the plan tables. Returns [B,256,256] fp32."""
    plan = build_plan()
    bf = ml_dtypes.bfloat16
    h = (plan["IRm"].astype(np.float64).T @ kern.astype(np.float64))  # [512]
    h2 = np.concatenate([h, h])
    Cm = np.zeros((T, T))
    for s in range(T):
        Cm[s] = h2[512 - s:1024 - s]
    Cm16 = Cm.astype(bf).astype(np.float64)
    sin16 = sinos.astype(bf).astype(np.float64)
    filt = np.einsum('bps,st->bpt', sin16, Cm16)
    filt16 = filt.astype(bf)  # DRAM filt [phi, t, b] bf16
    filtf = filt16.astype(np.float64)

    out = np.zeros((sinos.shape[0], H, W))
    for c in range(NCORE):
        WA = plan["WAs"][c].astype(np.float64)
        WB = plan["WBs"][c].astype(np.float64)
        for j, tau in enumerate(plan["core_nus"][c]):
            for gi in range(NG):
                s = plan["signs"][gi]
                aa = plan["inv_a"][gi, :NCLS]
                m = plan["gT"][gi][tau]
                pm = plan["gP"][gi]
                acc = np.zeros((sinos.shape[0], TS * TS))
                ti, tj = divmod(int(tau), NT)
                iiv = np.arange(TS, dtype=np.float64)
                b0 = np.zeros(NCLS, np.int64)
                for cc in range(NCLS):
                    Ac, Bcc, Ccc = _uABC(cc)
                    u = Ac * (ti * TS + iiv[:, None]) + Bcc * (tj * TS + iiv[None, :]) + Ccc
                    b0[cc] = np.clip(int(np.floor(u.min())), 0, T - KWIN)
                mb = b0 if s == 1 else (T - KWIN) - b0
                for cc in range(NCLS):
                    g = filtf[:, aa[cc], mb[cc]:mb[cc] + KWIN]     # [B, 36]
                    if s == -1:
                        g = g[:, ::-1]   # member row k reads tap mb + (35-k)
                    Wk = (WA[j, cc] if cc < 128 else WB[j, cc - 128])  # [36, 256]
                    acc += g @ Wk
                accp = np.zeros_like(acc)
                accp[:, pm] = acc
                mi, mj = divmod(int(m), NT)
                out[:, mi*TS:(mi+1)*TS, mj*TS:(mj+1)*TS] += \
                    accp.reshape(-1, TS, TS)
    return out.astype(np.float32)



# ======================================================================
# Device program
# ======================================================================
TAP_ROWS = PHI * T            # 368,640 tap-rows of B elems in filt_full
BAND_ELEMS = NCLS * T * B     # 2,949,120 elements per angle band
FILT_ELEMS = PHI * T * B      # 11,796,480
GROW = GWIN * B               # 1024 elements per gathered window

_nc_cache = None

def _build_nc():
    global _nc_cache
    if _nc_cache is not None:
        return _nc_cache
    import concourse.bass as bass
    import concourse.bacc as bacc
    import concourse.mybir as mybir
    import concourse.tile as tile
    from contextlib import ExitStack

    plan = build_plan()
    signs = plan["signs"]; bands = plan["bands"]
    bf = mybir.dt.bfloat16
    f8 = mybir.dt.float8e4
    f32 = mybir.dt.float32

    nc = bacc.Bacc(None, target_bir_lowering=False)
    sinoT = nc.dram_tensor("sinoT", [4, 128, APC * B], bf, kind="ExternalInput")
    kern = nc.dram_tensor("kern", [384, 1], f32, kind="ExternalInput")
    irm = nc.dram_tensor("irm", [384, 512], f32, kind="ExternalInput")
    coffs = nc.dram_tensor("coffs", [4, 128, 1], mybir.dt.int32, kind="ExternalInput")
    wa = nc.dram_tensor("wa", [NU_PER_CORE, 128, KWIN, 256], bf, kind="ExternalInput")
    wb = nc.dram_tensor("wb", [NU_PER_CORE, 52, KWIN, 256], bf, kind="ExternalInput")
    gidx = nc.dram_tensor("gidx", [NU_PER_CORE, 2, 128, NG], mybir.dt.int32,
                          kind="ExternalInput")
    oslots = nc.dram_tensor("oslots", [NU_PER_CORE, 2, 128, 256], f32,
                            kind="ExternalOutput")
    import os as _os
    _dump = _os.environ.get("FBP_DUMP_FILT") == "1"
    if _dump:
        fdump = nc.dram_tensor("fdump", [FILT_ELEMS // 128, 128], bf,
                               kind="ExternalOutput")
    h2d = nc.dram_tensor("h2d", [1024, 1], f32)
    HALF = (APC // 2) * T * B          # 45 angles per AllGather half
    filt_a = nc.dram_tensor("filt_a", [HALF], bf)
    filt_b = nc.dram_tensor("filt_b", [HALF], bf)
    filt_full = nc.dram_tensor("filt_full", [FILT_ELEMS + 256], bf,
                               addr_space="Shared")

    with tile.TileContext(nc) as tc:
        # ---------------- filter phase ----------------
        with (tc.tile_pool(name="fsb", bufs=1) as fsb,
              tc.tile_pool(name="fwork", bufs=2) as fwork,
              tc.tile_pool(name="fps", bufs=2, space="PSUM") as fps):
            # F0: h = IRm.T @ kern
            # float32r (not fp32) keeps walrus's FWL enabled for all later
            # bf16 LDWEIGHTS (EnableFWL requires no preceding fp32-HI matmul)
            f32r = mybir.dt.float32r
            hps = fps.tile([1, 512], f32, tag="hps")
            for ch, (k0, ksz) in enumerate(((0, 128), (128, 128), (256, 128))):
                kt = fsb.tile([ksz, 1], f32, tag=f"kt{ch}")
                nc.sync.dma_start(kt[:], kern[k0:k0 + ksz, :])
                irt = fsb.tile([ksz, 512], f32, tag=f"irt{ch}")
                nc.sync.dma_start(irt[:], irm[k0:k0 + ksz, :])
                ktr = fsb.tile([ksz, 1], f32r, tag=f"ktr{ch}")
                nc.vector.tensor_copy(ktr[:], kt[:])
                irtr = fsb.tile([ksz, 512], f32r, tag=f"irtr{ch}")
                nc.vector.tensor_copy(irtr[:], irt[:])
                nc.tensor.matmul(hps[:], lhsT=ktr[:], rhs=irtr[:],
                                 start=(ch == 0), stop=(ch == 2))
            h2sb = fsb.tile([1, 1024], f32, tag="h2sb")
            nc.vector.tensor_copy(h2sb[:, 0:512], hps[:])
            nc.vector.tensor_copy(h2sb[:, 512:1024], hps[:])
            nc.sync.dma_start(h2d[:].rearrange("a b -> b a"), h2sb[:])

            # F1: circulant chunks C[ch] = h2[512 - s + t], bf16
            csb = []
            for ch in range(4):
                co = fsb.tile([128, 1], mybir.dt.int32, tag=f"co{ch}")
                nc.sync.dma_start(co[:], coffs[ch])
                cf = fsb.tile([128, 512], f32, tag=f"cf{ch}")
                nc.gpsimd.indirect_dma_start(
                    out=cf[:], out_offset=None, in_=h2d[:],
                    in_offset=bass.IndirectOffsetOnAxis(ap=co[:], axis=0))
                cb = fsb.tile([128, 512], bf, tag=f"cb{ch}")
                nc.vector.tensor_copy(cb[:], cf[:])
                csb.append(cb)

            # F2/F3: transposed filter: psum[t, (phi,b)] = C[s,t].T @ sinoT[s,(phi,b)]
            # 6 chunks of 15 angles; AllGather each 45-angle half as soon as
            # its stores land so AG(half 0) overlaps filtering of half 1.
            st_all = []
            for ch in range(4):
                st = fsb.tile([128, APC * B], bf, tag=f"st{ch}")
                nc.sync.dma_start(st[:], sinoT[ch])
                st_all.append(st)
            ftens = filt_full.tensor if hasattr(filt_full, "tensor") else filt_full
            CSZ = 15 * B                       # 480 columns per chunk
            for half, floc in ((0, filt_a), (1, filt_b)):
                fltens = floc.tensor if hasattr(floc, "tensor") else floc
                for ci in range(3):
                    cs = (half * 3 + ci) * CSZ
                    nphi = 15
                    for tch in range(4):
                        fp = fps.tile([128, 512], f32, tag="fp")
                        for ch in range(4):
                            nc.tensor.matmul(
                                fp[:, :CSZ],
                                lhsT=csb[ch][:, tch * 128:(tch + 1) * 128],
                                rhs=st_all[ch][:, cs:cs + CSZ],
                                start=(ch == 0), stop=(ch == 3))
                        fb = fwork.tile([128, 512], bf, tag="fb")
                        if tch % 2 == 0:
                            nc.vector.tensor_copy(fb[:, :CSZ], fp[:, :CSZ])
                        else:
                            nc.scalar.copy(fb[:, :CSZ], fp[:, :CSZ])
                        # store: partition p = t (tch*128+p); free = (phi_rel, b)
                        out_ap = bass.AP(fltens, ci * 15 * (T * B) + tch * 128 * B,
                                         [[B, 128], [T * B, nphi], [1, B]])
                        nc.sync.dma_start(out_ap,
                                          fb[:, :CSZ].rearrange("p (f b) -> p f b", b=B))
                # F4: AllGather of this half; filt_full layout is
                # [half, rank, 45, t, b] (gather offsets encode it)
                nc.gpsimd.collective_compute(
                    "AllGather", mybir.AluOpType.bypass,
                    replica_groups=[list(range(NCORE))],
                    ins=[floc[:]],
                    outs=[bass.AP(ftens, half * NCORE * HALF,
                                  [[1, NCORE * HALF]])],
                )

        if _dump:
            with tc.tile_pool(name="fd", bufs=2) as fd:
                ftens0 = filt_full.tensor if hasattr(filt_full, "tensor") else filt_full
                CH = FILT_ELEMS // 128 // 32
                for i in range(32):
                    td = fd.tile([128, CH], bf, tag="td")
                    nc.sync.dma_start(td[:], bass.AP(ftens0, i * 128 * CH,
                                                     [[CH, 128], [1, CH]]))
                    nc.sync.dma_start(fdump[:].rearrange("(i p) c -> i p c", i=32)[i], td[:])
        # ---------------- backprojection ----------------
        # Fused layout: one gab tile [cls, 2 cch, 8 slot, KWIN, B] per nu.
        # Slots 0-3 = sign+1 groups (bands 0..3), 4-7 = sign-1 (bands 0..3).
        # lhsT [cls, (4 slots, 32 b)] = M=128 per matmul -> 128 matmuls/nu.
        with (tc.tile_pool(name="bsb", bufs=1) as bsb,
              tc.tile_pool(name="bg", bufs=2) as bg,
              tc.tile_pool(name="bgf", bufs=2) as bgf,
              tc.tile_pool(name="bw", bufs=2) as bw,
              tc.tile_pool(name="bst", bufs=2) as bst,
              tc.tile_pool(name="bps", bufs=2, space="PSUM") as bps):
            idxsb = bsb.tile([128, NU_PER_CORE, 2, NG], mybir.dt.int32, tag="idx")
            nc.sync.dma_start(idxsb[:], gidx[:].transpose([2, 0, 1, 3]))
            ftens = filt_full.tensor if hasattr(filt_full, "tensor") else filt_full
            otens = oslots.tensor if hasattr(oslots, "tensor") else oslots
            filt_rows = bass.AP(ftens, 0, [[B, TAP_ROWS], [1, B]])
            for nu in range(NU_PER_CORE):
                # per (cch, slot) indirect DMA: one offset per partition
                # (class), each transferring a contiguous 32-tap window
                gab = bg.tile([128, 2, NG, KWIN, B], bf, tag="gab")
                for cch, ksz in ((0, 128), (1, 52)):
                    for slot in range(NG):
                        nc.gpsimd.indirect_dma_start(
                            out=gab[0:ksz, cch, slot].rearrange("p k b -> p (k b)"),
                            out_offset=None,
                            in_=filt_rows,
                            in_offset=bass.IndirectOffsetOnAxis(
                                ap=idxsb[0:ksz, nu, cch, slot:slot + 1], axis=0))
                # repack to [cls, gh, cch, tap, (gi4 b)] so each lhsT slice
                # is a single flat 128-wide free dim (keeps walrus FWL on;
                # PE stationary APs allow only one free dimension)
                gf = bgf.tile([128, 2, 2, KWIN, 4 * B], bf, tag="gf")
                for slot in range(NG):
                    gh, g4 = divmod(slot, 4)
                    dst = gf[:, gh, :, :, g4 * B:(g4 + 1) * B]
                    if slot % 4 != 3:
                        nc.vector.tensor_copy(dst, gab[:, :, slot])
                    else:
                        nc.scalar.copy(dst, gab[:, :, slot])
                wat = bw.tile([128, KWIN, 256], bf, tag="wa")
                nc.sync.dma_start(wat[:], wa[nu])
                wbt = bw.tile([52, KWIN, 256], bf, tag="wb")
                nc.sync.dma_start(wbt[:], wb[nu])
                ps0 = bps.tile([128, 256], f32, tag="ps0")
                ps1 = bps.tile([128, 256], f32, tag="ps1")
                pst = [ps0, ps1]
                for cchunk in range(2):
                    for k in range(KWIN):
                        for gh in range(2):
                            ks = k if gh == 0 else KWIN - 1 - k
                            if cchunk == 0:
                                lhs = gf[:, gh, 0, ks, :]
                                rhs = wat[:, k, :]
                            else:
                                lhs = gf[0:52, gh, 1, ks, :]
                                rhs = wbt[:, k, :]
                            nc.tensor.matmul(
                                pst[gh][:],
                                lhsT=lhs, rhs=rhs,
                                start=(cchunk == 0 and k == 0),
                                stop=(cchunk == 1 and k == KWIN - 1),
                            )
                for gh in range(2):
                    stg = bst.tile([128, 256], f32, tag="stg")
                    if gh == 0:
                        nc.vector.tensor_copy(stg[:], pst[gh][:])
                    else:
                        nc.scalar.copy(stg[:], pst[gh][:])
                    out_ap = bass.AP(otens,
                                     (nu * 2 + gh) * 128 * 256,
                                     [[256, 128], [1, 256]])
                    nc.sync.dma_start(out_ap, stg[:])
    nc.compile()
    _nc_cache = nc
    return nc


def _prep_inputs(sinos, kern_in):
    plan = build_plan()
    bf = ml_dtypes.bfloat16
    in_maps = []
    kern_t = np.zeros((384, 1), np.float32)
    kern_t[:257, 0] = np.asarray(kern_in, np.float32)
    irm_pad = np.zeros((384, 512), np.float32)
    irm_pad[:257] = plan["IRm"]
    for c in range(NCORE):
        sl = np.asarray(sinos[:, c * APC:(c + 1) * APC, :], np.float64)
        st = sl.transpose(2, 1, 0).reshape(T, APC * B)   # [512, (phi_rel, b)]
        st = st.reshape(4, 128, APC * B).astype(bf)
        in_maps.append({
            "sinoT": st,
            "kern": kern_t,
            "irm": irm_pad,
            "coffs": plan["coffs"],
            "wa": plan["WAs"][c],
            "wb": plan["WBs"][c],
            "gidx": plan["GIs"][c],
        })
    return in_maps


def _merge_outputs(results):
    plan = build_plan()
    out = np.zeros((B, H, W), np.float64)
    for c in range(NCORE):
        # [16, 2, 128, 256] -> [16, 8(slot), 32, 256]
        slots = results[c]["oslots"].astype(np.float64).reshape(
            NU_PER_CORE, NG, B, 256)
        for j in range(NU_PER_CORE):
            for slot in range(NG):
                m = int(plan["merges"][c][j, slot, 0])
                gi = int(plan["merges"][c][j, slot, 1])
                pm = plan["gP"][gi]
                accp = np.zeros((B, TS * TS))
                accp[:, pm] = slots[j, slot]
                mi, mj = divmod(m, NT)
                out[:, mi * TS:(mi + 1) * TS, mj * TS:(mj + 1) * TS] += \
                    accp.reshape(B, TS, TS)
    return out.astype(np.float32)


def kernel(sinos, kernel):
    from concourse.bass_utils import run_bass_kernel_spmd
    sinos = np.asarray(sinos)
    kern_in = np.asarray(kernel)
    nc = _build_nc()
    in_maps = _prep_inputs(sinos, kern_in)
    res = run_bass_kernel_spmd(nc, in_maps, list(range(NCORE)))
    return _merge_outputs(res.results)

